# revision 11
# baseline (speedup 1.0000x reference)
"""Trainium2 Bass kernel for nn_DistillLoss (ragged KL distillation loss).

Strategy (data-parallel over batch, 8 NeuronCores):
  - Host: shard B=1024 samples into 8 x 128, pad each sample's ragged doc
    segment to a dense [128 samples, M=128 docs, D=1024] block per core,
    converted to bf16 (zeros in the padding -> sim contribution is 0 and
    masked anyway).
  - Device (per core): stream doc groups [128, grp, 1024] bf16 from HBM,
    alternating between the two HWDGE queues (qSyncDynamicHW /
    qScalarDynamicHW) so both rings pull concurrently.  For each doc slot
    m, one DVE scalar_tensor_tensor in bf16 (2x packed mode) computes
    sim[b, m] = (docs[b,m,:] . q[b,:]) / TEMP into an fp32 accumulator.
    Masked log-softmax + KL epilogue runs on [b=128, m=128] fp32 tiles.
    Each core emits one partial scalar; host sums and divides by B.
"""

import sys

sys.path.insert(0, "/opt/trn_rl_repo")

import numpy as np

NCORES = 8
B = 1024
D = 1024
M = 128
BL = B // NCORES  # 128 samples per core
TEMP = 0.02
NEG = -1e30
GRP = 4  # doc slots per DMA group ([128, 4, 1024] bf16 = 1 MB per dma_start)

_CACHE = {}


def _build_nc(grp=8, dbufs=4, pool_stt=0, nq=2, ttr=False):
    from concourse import bacc, bass_isa, mybir, tile

    f32 = mybir.dt.float32
    bf16 = mybir.dt.bfloat16
    u8 = mybir.dt.uint8
    ALU = mybir.AluOpType
    AF = mybir.ActivationFunctionType
    AX = mybir.AxisListType

    nc = bacc.Bacc("TRN2", target_bir_lowering=False, debug=False, num_devices=NCORES)

    docs = nc.dram_tensor("docs", [BL * M, D], bf16, kind="ExternalInput").ap()
    q = nc.dram_tensor("q", [BL, D], bf16, kind="ExternalInput").ap()
    traw = nc.dram_tensor("traw", [BL, M], f32, kind="ExternalInput").ap()
    mask = nc.dram_tensor("mask", [BL, M], u8, kind="ExternalInput").ap()
    out = nc.dram_tensor("out", [1, 1], f32, kind="ExternalOutput").ap()

    from contextlib import ExitStack

    with tile.TileContext(nc) as tc, ExitStack() as ctx:
        consts = ctx.enter_context(tc.tile_pool(name="consts", bufs=1))
        dpool = ctx.enter_context(tc.tile_pool(name="docs", bufs=dbufs))
        scratch = ctx.enter_context(tc.tile_pool(name="scratch", bufs=2))
        small = ctx.enter_context(tc.tile_pool(name="small", bufs=1))

        traw_sb = consts.tile([BL, M], f32)
        nc.scalar.dma_start(out=traw_sb, in_=traw)
        mask_sb = consts.tile([BL, M], u8)
        nc.scalar.dma_start(out=mask_sb, in_=mask)
        q_sb = consts.tile([BL, D], bf16)
        nc.sync.dma_start(out=q_sb, in_=q)
        negt = consts.tile([BL, M], f32)
        nc.vector.memset(negt, NEG)
        sim_bm = consts.tile([BL, M], f32)  # [b, m] raw logits

        # docs rows laid out (b, m); tile over m with b on partitions.
        docs_bm = docs.rearrange("(b m) d -> b m d", m=M)

        qeng = [nc.sync, nc.scalar, nc.gpsimd][:nq]
        for g in range(M // grp):
            dtile = dpool.tile([BL, grp, D], bf16)
            qeng[g % nq].dma_start(
                out=dtile, in_=docs_bm[:, g * grp : (g + 1) * grp, :]
            )
            for j in range(grp):
                m = g * grp + j
                sc = scratch.tile([BL, D], bf16, tag="sc")
                if ttr:
                    nc.vector.tensor_tensor_reduce(
                        out=sc,
                        in0=dtile[:, j, :],
                        in1=q_sb,
                        scale=1.0 / TEMP,
                        scalar=0.0,
                        op0=ALU.mult,
                        op1=ALU.add,
                        accum_out=sim_bm[:, m : m + 1],
                    )
                else:
                    nc.vector.scalar_tensor_tensor(
                        out=sc,
                        in0=dtile[:, j, :],
                        scalar=1.0 / TEMP,
                        in1=q_sb,
                        op0=ALU.mult,
                        op1=ALU.mult,
                        accum_out=sim_bm[:, m : m + 1],
                    )

        # ---- epilogue on [b=128, m=128] fp32 tiles ----
        simm = small.tile([BL, M], f32)
        nc.vector.select(simm, mask_sb, sim_bm, negt)

        nmx = small.tile([BL, 1], f32)
        nc.vector.tensor_reduce(nmx, simm, axis=AX.X, op=ALU.max, negate=True)
        shifted = small.tile([BL, M], f32)
        nc.vector.tensor_scalar_add(shifted, simm, nmx[:, 0:1])

        e_sb = small.tile([BL, M], f32)
        s_sb = small.tile([BL, 1], f32)
        nc.scalar.activation(e_sb, shifted, AF.Exp, accum_out=s_sb)
        logs = small.tile([BL, 1], f32)
        nc.scalar.activation(logs, s_sb, AF.Ln)

        tsum = small.tile([BL, 1], f32)
        nc.vector.tensor_reduce(tsum, traw_sb, axis=AX.X, op=ALU.add)
        denom = small.tile([BL, 1], f32)
        nc.vector.tensor_scalar_add(denom, tsum, 1e-9)
        rec = small.tile([BL, 1], f32)
        nc.vector.reciprocal(rec, denom)
        tn = small.tile([BL, M], f32)
        nc.vector.tensor_scalar_mul(tn, traw_sb, rec[:, 0:1])
        sumtn = small.tile([BL, 1], f32)
        nc.vector.tensor_mul(sumtn, tsum, rec)

        iszero = small.tile([BL, M], f32)
        nc.vector.tensor_scalar(iszero, tn, 0.0, None, op0=ALU.is_le)
        tsafe = small.tile([BL, M], f32)
        nc.vector.tensor_add(tsafe, tn, iszero)
        logt = small.tile([BL, M], f32)
        nc.scalar.activation(logt, tsafe, AF.Ln)

        sc2 = small.tile([BL, M], f32)
        term1 = small.tile([BL, 1], f32)
        nc.vector.scalar_tensor_tensor(
            out=sc2, in0=tn, scalar=1.0, in1=logt,
            op0=ALU.mult, op1=ALU.mult, accum_out=term1,
        )
        sc3 = small.tile([BL, M], f32)
        term2 = small.tile([BL, 1], f32)
        nc.vector.scalar_tensor_tensor(
            out=sc3, in0=tn, scalar=1.0, in1=shifted,
            op0=ALU.mult, op1=ALU.mult, accum_out=term2,
        )

        lgs = small.tile([BL, 1], f32)
        nc.vector.tensor_mul(lgs, logs, sumtn)
        kc = small.tile([BL, 1], f32)
        nc.vector.tensor_sub(kc, term1, term2)
        nc.vector.tensor_add(kc, kc, lgs)

        tot = small.tile([128, 1], f32)
        nc.gpsimd.partition_all_reduce(
            tot, kc, channels=128, reduce_op=bass_isa.ReduceOp.add
        )
        nc.sync.dma_start(out=out, in_=tot[0:1, 0:1])

    nc.compile()
    return nc


def _build_nc_pe(widths, dbufs=6, nq=3, gelems=4096):
    """PE-route: per (sample-slot, d-chunk) matmul with a 1-column stationary
    (the sample's q chunk) and the sample's transposed doc chunk as the moving
    tensor, accumulating sim[slot, :] in PSUM over the 8 d-chunks.

    widths[i]: static doc count for slot i (max over cores of the slot's
    sample nd after sorting+dealing); docs arrive pre-transposed/packed as a
    [128 dpos, TOT] bf16 slab, TOT = sum(8 * w_i).
    """
    from concourse import bacc, bass_isa, mybir, tile

    f32 = mybir.dt.float32
    bf16 = mybir.dt.bfloat16
    u8 = mybir.dt.uint8
    ALU = mybir.AluOpType
    AF = mybir.ActivationFunctionType
    AX = mybir.AxisListType
    NCH = D // 128  # 8 d-chunks

    widths = list(widths)
    offs = [0]
    for w in widths:
        offs.append(offs[-1] + NCH * w)
    TOT = offs[-1]

    nc = bacc.Bacc("TRN2", target_bir_lowering=False, debug=False, num_devices=NCORES)

    docs = nc.dram_tensor("docs", [128, TOT], bf16, kind="ExternalInput").ap()
    qt = nc.dram_tensor("qt", [128, NCH * BL], bf16, kind="ExternalInput").ap()
    traw = nc.dram_tensor("traw", [BL, M], f32, kind="ExternalInput").ap()
    mask = nc.dram_tensor("mask", [BL, M], u8, kind="ExternalInput").ap()
    out = nc.dram_tensor("out", [1, 1], f32, kind="ExternalOutput").ap()

    # group consecutive slots into DMA chunks of <= gelems elements/partition
    groups = []
    a = 0
    while a < BL:
        b = a
        while b < BL and offs[b + 1] - offs[a] <= gelems:
            b += 1
        groups.append((a, b))
        a = b

    from contextlib import ExitStack

    with tile.TileContext(nc) as tc, ExitStack() as ctx:
        consts = ctx.enter_context(tc.tile_pool(name="consts", bufs=1))
        dpool = ctx.enter_context(tc.tile_pool(name="docs", bufs=dbufs))
        small = ctx.enter_context(tc.tile_pool(name="small", bufs=1))
        psum = ctx.enter_context(tc.tile_pool(name="psum", bufs=1, space="PSUM"))

        traw_sb = consts.tile([BL, M], f32)
        nc.scalar.dma_start(out=traw_sb, in_=traw)
        mask_sb = consts.tile([BL, M], u8)
        nc.scalar.dma_start(out=mask_sb, in_=mask)
        qt_sb = consts.tile([128, NCH * BL], bf16)
        nc.sync.dma_start(out=qt_sb, in_=qt)
        negt = consts.tile([BL, M], f32)
        nc.vector.memset(negt, NEG)

        # sim accumulates TRANSPOSED: psimT[m, slot] (matmul out base partition
        # must be 0, so slots live on the free axis).
        psimT = psum.tile([M, BL], f32)
        nc.vector.memset(psimT, 0.0)

        qeng = [nc.sync, nc.scalar, nc.gpsimd][:nq]
        for g, (a, b) in enumerate(groups):
            sz = offs[b] - offs[a]
            dtile = dpool.tile([128, sz], bf16)
            qeng[g % nq].dma_start(out=dtile, in_=docs[:, offs[a] : offs[b]])
            for i in range(a, b):
                w = widths[i]
                o = offs[i] - offs[a]
                for dc in range(NCH):
                    nc.tensor.matmul(
                        out=psimT[0:w, i : i + 1],
                        lhsT=dtile[:, o + dc * w : o + (dc + 1) * w],
                        rhs=qt_sb[:, dc * BL + i : dc * BL + i + 1],
                        start=(dc == 0),
                        stop=(dc == NCH - 1),
                    )

        # ---- epilogue on [b=128, m=128] fp32 tiles ----
        # full transpose psimT -> sim_raw via 16 DVE 32x32 block transposes
        SQ = 32
        sim_raw = small.tile([BL, M], f32)
        for bi in range(M // SQ):
            for bj in range(BL // SQ):
                nc.vector.transpose(
                    out=sim_raw[bj * SQ : (bj + 1) * SQ, bi * SQ : (bi + 1) * SQ],
                    in_=psimT[bi * SQ : (bi + 1) * SQ, bj * SQ : (bj + 1) * SQ],
                )
        sim_bm = small.tile([BL, M], f32)
        nc.vector.tensor_scalar(sim_bm, sim_raw, 1.0 / TEMP, None, op0=ALU.mult)
        simm = small.tile([BL, M], f32)
        nc.vector.select(simm, mask_sb, sim_bm, negt)

        nmx = small.tile([BL, 1], f32)
        nc.vector.tensor_reduce(nmx, simm, axis=AX.X, op=ALU.max, negate=True)
        shifted = small.tile([BL, M], f32)
        nc.vector.tensor_scalar_add(shifted, simm, nmx[:, 0:1])

        e_sb = small.tile([BL, M], f32)
        s_sb = small.tile([BL, 1], f32)
        nc.scalar.activation(e_sb, shifted, AF.Exp, accum_out=s_sb)
        logs = small.tile([BL, 1], f32)
        nc.scalar.activation(logs, s_sb, AF.Ln)

        tsum = small.tile([BL, 1], f32)
        nc.vector.tensor_reduce(tsum, traw_sb, axis=AX.X, op=ALU.add)
        denom = small.tile([BL, 1], f32)
        nc.vector.tensor_scalar_add(denom, tsum, 1e-9)
        rec = small.tile([BL, 1], f32)
        nc.vector.reciprocal(rec, denom)
        tn = small.tile([BL, M], f32)
        nc.vector.tensor_scalar_mul(tn, traw_sb, rec[:, 0:1])
        sumtn = small.tile([BL, 1], f32)
        nc.vector.tensor_mul(sumtn, tsum, rec)

        iszero = small.tile([BL, M], f32)
        nc.vector.tensor_scalar(iszero, tn, 0.0, None, op0=ALU.is_le)
        tsafe = small.tile([BL, M], f32)
        nc.vector.tensor_add(tsafe, tn, iszero)
        logt = small.tile([BL, M], f32)
        nc.scalar.activation(logt, tsafe, AF.Ln)

        sc2 = small.tile([BL, M], f32)
        term1 = small.tile([BL, 1], f32)
        nc.vector.scalar_tensor_tensor(
            out=sc2, in0=tn, scalar=1.0, in1=logt,
            op0=ALU.mult, op1=ALU.mult, accum_out=term1,
        )
        sc3 = small.tile([BL, M], f32)
        term2 = small.tile([BL, 1], f32)
        nc.vector.scalar_tensor_tensor(
            out=sc3, in0=tn, scalar=1.0, in1=shifted,
            op0=ALU.mult, op1=ALU.mult, accum_out=term2,
        )

        lgs = small.tile([BL, 1], f32)
        nc.vector.tensor_mul(lgs, logs, sumtn)
        kc = small.tile([BL, 1], f32)
        nc.vector.tensor_sub(kc, term1, term2)
        nc.vector.tensor_add(kc, kc, lgs)

        tot = small.tile([128, 1], f32)
        nc.gpsimd.partition_all_reduce(
            tot, kc, channels=128, reduce_op=bass_isa.ReduceOp.add
        )
        nc.sync.dma_start(out=out, in_=tot[0:1, 0:1])

    nc.compile()
    return nc


def _get_nc(**cfg):
    key = ("nc",) + tuple(sorted(cfg.items()))
    if key not in _CACHE:
        c = dict(cfg)
        if "widths" in c:
            _CACHE[key] = _build_nc_pe(**c)
        else:
            _CACHE[key] = _build_nc(**c)
    return _CACHE[key]


def _make_in_maps_pe(query_embeds, doc_embeds, soft_labels, num_docs_per_sample):
    """Sort samples by doc count, deal round-robin to cores so slot i holds
    near-equal nd everywhere; slot width = max nd over cores at that slot.
    Docs packed per core as a [128 dpos, TOT] bf16 slab: per slot, 8
    transposed d-chunk tiles [128, w] back to back."""
    import ml_dtypes

    bf16 = ml_dtypes.bfloat16
    NCH = D // 128
    qf = np.asarray(query_embeds, dtype=np.float32)
    de = np.asarray(doc_embeds, dtype=np.float32)
    sl = np.ascontiguousarray(np.asarray(soft_labels, dtype=np.float32))
    nd = np.asarray(num_docs_per_sample).astype(np.int64)
    total = de.shape[0]

    offs = np.zeros(B, np.int64)
    offs[1:] = np.cumsum(nd)[:-1]
    nde = np.minimum(np.minimum(nd, M), np.maximum(total - offs, 0))
    maskf = (np.arange(M)[None, :] < nde[:, None]).astype(np.float32)
    traw = sl * maskf

    order = np.argsort(nde, kind="stable")  # ascending nd
    # slot i on core c processes sample order[i*NCORES + c]
    sl_mat = order.reshape(BL, NCORES)
    widths = nde[sl_mat].max(axis=1).astype(np.int64)  # [BL], static per slot
    woffs = np.zeros(BL + 1, np.int64)
    woffs[1:] = np.cumsum(NCH * widths)
    TOT = int(woffs[-1])

    de16 = de.astype(bf16)
    q16 = qf.astype(bf16)

    in_maps = []
    for c in range(NCORES):
        samp = sl_mat[:, c]  # [BL] sample ids for this core, slot order
        slab = np.zeros((128, TOT), bf16)
        qt = np.zeros((128, NCH * BL), bf16)
        for i in range(BL):
            s = int(samp[i])
            n = int(nde[s])
            w = int(widths[i])
            o = int(offs[s])
            # [n, D] -> [D, n] -> [NCH, 128, n]
            t3 = np.ascontiguousarray(de16[o : o + n].T).reshape(NCH, 128, n)
            base = int(woffs[i])
            for dc in range(NCH):
                slab[:, base + dc * w : base + dc * w + n] = t3[dc]
            qt[:, i::BL] = q16[s].reshape(NCH, 128).T  # qt[dpos, dc*BL+i]
        in_maps.append(
            {
                "docs": slab,
                "qt": qt,
                "traw": np.ascontiguousarray(traw[samp]),
                "mask": np.ascontiguousarray(maskf[samp].astype(np.uint8)),
            }
        )
    return in_maps, {"widths": tuple(int(w) for w in widths)}


USE_PE = True


def _make_in_maps(query_embeds, doc_embeds, soft_labels, num_docs_per_sample):
    if USE_PE:
        return _make_in_maps_pe(
            query_embeds, doc_embeds, soft_labels, num_docs_per_sample
        )
    return _make_in_maps_dense(
        query_embeds, doc_embeds, soft_labels, num_docs_per_sample
    )


def _make_in_maps_dense(query_embeds, doc_embeds, soft_labels, num_docs_per_sample):
    import ml_dtypes

    bf16 = ml_dtypes.bfloat16
    qf = np.asarray(query_embeds, dtype=np.float32)
    de = np.asarray(doc_embeds, dtype=np.float32)
    sl = np.ascontiguousarray(np.asarray(soft_labels, dtype=np.float32))
    nd = np.asarray(num_docs_per_sample).astype(np.int64)
    total = de.shape[0]

    offs = np.zeros(B, np.int64)
    offs[1:] = np.cumsum(nd)[:-1]
    # effective (clipped) doc counts, mirroring the reference's clip behaviour
    nde = np.minimum(np.minimum(nd, M), np.maximum(total - offs, 0))
    mask = (np.arange(M)[None, :] < nde[:, None]).astype(np.float32)
    traw = sl * mask

    docs_pad = np.zeros((B, M, D), bf16)
    de16 = de.astype(bf16)
    for b in range(B):
        n = int(nde[b])
        o = int(offs[b])
        if n > 0:
            docs_pad[b, :n] = de16[o : o + n]
    q16 = qf.astype(bf16)

    in_maps = []
    for c in range(NCORES):
        s = slice(c * BL, (c + 1) * BL)
        in_maps.append(
            {
                "docs": np.ascontiguousarray(docs_pad[s].reshape(BL * M, D)),
                "q": np.ascontiguousarray(q16[s]),
                "traw": np.ascontiguousarray(traw[s]),
                "mask": np.ascontiguousarray(mask[s].astype(np.uint8)),
            }
        )
    return in_maps, {}


def run(in_maps, cfg=None, trace=False):
    from concourse import bass_utils

    nc = _get_nc(**(cfg or {}))
    return bass_utils.run_bass_kernel_spmd(
        nc, in_maps, list(range(NCORES)), trace=trace
    )


def kernel(query_embeds, doc_embeds, soft_labels, num_docs_per_sample):
    in_maps, cfg = _make_in_maps(
        query_embeds, doc_embeds, soft_labels, num_docs_per_sample
    )
    res = run(in_maps, cfg=cfg)
    tot = sum(float(r["out"][0, 0]) for r in res.results)
    return np.asarray(tot / B, dtype=np.float32)


# revision 12
# speedup vs baseline: 1.1951x; 1.1951x over previous
"""Trainium2 Bass kernel for nn_DistillLoss (ragged KL distillation loss).

Strategy (data-parallel over batch, 8 NeuronCores):
  - Host: shard B=1024 samples into 8 x 128, pad each sample's ragged doc
    segment to a dense [128 samples, M=128 docs, D=1024] block per core,
    converted to bf16 (zeros in the padding -> sim contribution is 0 and
    masked anyway).
  - Device (per core): stream doc groups [128, grp, 1024] bf16 from HBM,
    alternating between the two HWDGE queues (qSyncDynamicHW /
    qScalarDynamicHW) so both rings pull concurrently.  For each doc slot
    m, one DVE scalar_tensor_tensor in bf16 (2x packed mode) computes
    sim[b, m] = (docs[b,m,:] . q[b,:]) / TEMP into an fp32 accumulator.
    Masked log-softmax + KL epilogue runs on [b=128, m=128] fp32 tiles.
    Each core emits one partial scalar; host sums and divides by B.
"""

import sys

sys.path.insert(0, "/opt/trn_rl_repo")

import numpy as np

NCORES = 8
B = 1024
D = 1024
M = 128
BL = B // NCORES  # 128 samples per core
TEMP = 0.02
NEG = -1e30
GRP = 4  # doc slots per DMA group ([128, 4, 1024] bf16 = 1 MB per dma_start)

_CACHE = {}


def _build_nc(grp=8, dbufs=4, pool_stt=0, nq=2, ttr=False):
    from concourse import bacc, bass_isa, mybir, tile

    f32 = mybir.dt.float32
    bf16 = mybir.dt.bfloat16
    u8 = mybir.dt.uint8
    ALU = mybir.AluOpType
    AF = mybir.ActivationFunctionType
    AX = mybir.AxisListType

    nc = bacc.Bacc("TRN2", target_bir_lowering=False, debug=False, num_devices=NCORES)

    docs = nc.dram_tensor("docs", [BL * M, D], bf16, kind="ExternalInput").ap()
    q = nc.dram_tensor("q", [BL, D], bf16, kind="ExternalInput").ap()
    traw = nc.dram_tensor("traw", [BL, M], f32, kind="ExternalInput").ap()
    mask = nc.dram_tensor("mask", [BL, M], u8, kind="ExternalInput").ap()
    out = nc.dram_tensor("out", [1, 1], f32, kind="ExternalOutput").ap()

    from contextlib import ExitStack

    with tile.TileContext(nc) as tc, ExitStack() as ctx:
        consts = ctx.enter_context(tc.tile_pool(name="consts", bufs=1))
        dpool = ctx.enter_context(tc.tile_pool(name="docs", bufs=dbufs))
        scratch = ctx.enter_context(tc.tile_pool(name="scratch", bufs=2))
        small = ctx.enter_context(tc.tile_pool(name="small", bufs=1))

        traw_sb = consts.tile([BL, M], f32)
        nc.scalar.dma_start(out=traw_sb, in_=traw)
        mask_sb = consts.tile([BL, M], u8)
        nc.scalar.dma_start(out=mask_sb, in_=mask)
        q_sb = consts.tile([BL, D], bf16)
        nc.sync.dma_start(out=q_sb, in_=q)
        negt = consts.tile([BL, M], f32)
        nc.vector.memset(negt, NEG)
        sim_bm = consts.tile([BL, M], f32)  # [b, m] raw logits

        # docs rows laid out (b, m); tile over m with b on partitions.
        docs_bm = docs.rearrange("(b m) d -> b m d", m=M)

        qeng = [nc.sync, nc.scalar, nc.gpsimd][:nq]
        for g in range(M // grp):
            dtile = dpool.tile([BL, grp, D], bf16)
            qeng[g % nq].dma_start(
                out=dtile, in_=docs_bm[:, g * grp : (g + 1) * grp, :]
            )
            for j in range(grp):
                m = g * grp + j
                sc = scratch.tile([BL, D], bf16, tag="sc")
                if ttr:
                    nc.vector.tensor_tensor_reduce(
                        out=sc,
                        in0=dtile[:, j, :],
                        in1=q_sb,
                        scale=1.0 / TEMP,
                        scalar=0.0,
                        op0=ALU.mult,
                        op1=ALU.add,
                        accum_out=sim_bm[:, m : m + 1],
                    )
                else:
                    nc.vector.scalar_tensor_tensor(
                        out=sc,
                        in0=dtile[:, j, :],
                        scalar=1.0 / TEMP,
                        in1=q_sb,
                        op0=ALU.mult,
                        op1=ALU.mult,
                        accum_out=sim_bm[:, m : m + 1],
                    )

        # ---- epilogue on [b=128, m=128] fp32 tiles ----
        simm = small.tile([BL, M], f32)
        nc.vector.select(simm, mask_sb, sim_bm, negt)

        nmx = small.tile([BL, 1], f32)
        nc.vector.tensor_reduce(nmx, simm, axis=AX.X, op=ALU.max, negate=True)
        shifted = small.tile([BL, M], f32)
        nc.vector.tensor_scalar_add(shifted, simm, nmx[:, 0:1])

        e_sb = small.tile([BL, M], f32)
        s_sb = small.tile([BL, 1], f32)
        nc.scalar.activation(e_sb, shifted, AF.Exp, accum_out=s_sb)
        logs = small.tile([BL, 1], f32)
        nc.scalar.activation(logs, s_sb, AF.Ln)

        tsum = small.tile([BL, 1], f32)
        nc.vector.tensor_reduce(tsum, traw_sb, axis=AX.X, op=ALU.add)
        denom = small.tile([BL, 1], f32)
        nc.vector.tensor_scalar_add(denom, tsum, 1e-9)
        rec = small.tile([BL, 1], f32)
        nc.vector.reciprocal(rec, denom)
        tn = small.tile([BL, M], f32)
        nc.vector.tensor_scalar_mul(tn, traw_sb, rec[:, 0:1])
        sumtn = small.tile([BL, 1], f32)
        nc.vector.tensor_mul(sumtn, tsum, rec)

        iszero = small.tile([BL, M], f32)
        nc.vector.tensor_scalar(iszero, tn, 0.0, None, op0=ALU.is_le)
        tsafe = small.tile([BL, M], f32)
        nc.vector.tensor_add(tsafe, tn, iszero)
        logt = small.tile([BL, M], f32)
        nc.scalar.activation(logt, tsafe, AF.Ln)

        sc2 = small.tile([BL, M], f32)
        term1 = small.tile([BL, 1], f32)
        nc.vector.scalar_tensor_tensor(
            out=sc2, in0=tn, scalar=1.0, in1=logt,
            op0=ALU.mult, op1=ALU.mult, accum_out=term1,
        )
        sc3 = small.tile([BL, M], f32)
        term2 = small.tile([BL, 1], f32)
        nc.vector.scalar_tensor_tensor(
            out=sc3, in0=tn, scalar=1.0, in1=shifted,
            op0=ALU.mult, op1=ALU.mult, accum_out=term2,
        )

        lgs = small.tile([BL, 1], f32)
        nc.vector.tensor_mul(lgs, logs, sumtn)
        kc = small.tile([BL, 1], f32)
        nc.vector.tensor_sub(kc, term1, term2)
        nc.vector.tensor_add(kc, kc, lgs)

        tot = small.tile([128, 1], f32)
        nc.gpsimd.partition_all_reduce(
            tot, kc, channels=128, reduce_op=bass_isa.ReduceOp.add
        )
        nc.sync.dma_start(out=out, in_=tot[0:1, 0:1])

    nc.compile()
    return nc


def _build_nc_pe(widths, dbufs=4, nq=2, gelems=8192):
    """PE-route: per (sample-slot, d-chunk) matmul with a 1-column stationary
    (the sample's q chunk) and the sample's transposed doc chunk as the moving
    tensor, accumulating sim[slot, :] in PSUM over the 8 d-chunks.

    widths[i]: static doc count for slot i (max over cores of the slot's
    sample nd after sorting+dealing); docs arrive pre-transposed/packed as a
    [128 dpos, TOT] bf16 slab, TOT = sum(8 * w_i).
    """
    from concourse import bacc, bass_isa, mybir, tile

    f32 = mybir.dt.float32
    bf16 = mybir.dt.bfloat16
    u8 = mybir.dt.uint8
    ALU = mybir.AluOpType
    AF = mybir.ActivationFunctionType
    AX = mybir.AxisListType
    NCH = D // 128  # 8 d-chunks

    widths = list(widths)
    offs = [0]
    for w in widths:
        offs.append(offs[-1] + NCH * w)
    TOT = offs[-1]

    nc = bacc.Bacc("TRN2", target_bir_lowering=False, debug=False, num_devices=NCORES)

    docs = nc.dram_tensor("docs", [128, TOT], bf16, kind="ExternalInput").ap()
    qt = nc.dram_tensor("qt", [128, NCH * BL], bf16, kind="ExternalInput").ap()
    traw = nc.dram_tensor("traw", [BL, M], f32, kind="ExternalInput").ap()
    mask = nc.dram_tensor("mask", [BL, M], u8, kind="ExternalInput").ap()
    out = nc.dram_tensor("out", [1, 1], f32, kind="ExternalOutput").ap()

    # group consecutive slots into DMA chunks of <= gelems elements/partition
    groups = []
    a = 0
    while a < BL:
        b = a
        while b < BL and offs[b + 1] - offs[a] <= gelems:
            b += 1
        groups.append((a, b))
        a = b

    from contextlib import ExitStack

    with tile.TileContext(nc) as tc, ExitStack() as ctx:
        consts = ctx.enter_context(tc.tile_pool(name="consts", bufs=1))
        dpool = ctx.enter_context(tc.tile_pool(name="docs", bufs=dbufs))
        small = ctx.enter_context(tc.tile_pool(name="small", bufs=1))
        psum = ctx.enter_context(tc.tile_pool(name="psum", bufs=1, space="PSUM"))

        traw_sb = consts.tile([BL, M], f32)
        nc.scalar.dma_start(out=traw_sb, in_=traw)
        mask_sb = consts.tile([BL, M], u8)
        nc.scalar.dma_start(out=mask_sb, in_=mask)
        qt_sb = consts.tile([128, NCH * BL], bf16)
        nc.sync.dma_start(out=qt_sb, in_=qt)
        negt = consts.tile([BL, M], f32)
        nc.vector.memset(negt, NEG)

        # sim accumulates TRANSPOSED: psimT[m, slot] (matmul out base partition
        # must be 0, so slots live on the free axis).
        psimT = psum.tile([M, BL], f32)
        nc.vector.memset(psimT, 0.0)

        qeng = [nc.sync, nc.scalar, nc.gpsimd][:nq]
        for g, (a, b) in enumerate(groups):
            sz = offs[b] - offs[a]
            dtile = dpool.tile([128, sz], bf16)
            qeng[g % nq].dma_start(out=dtile, in_=docs[:, offs[a] : offs[b]])
            for i in range(a, b):
                w = widths[i]
                o = offs[i] - offs[a]
                for dc in range(NCH):
                    nc.tensor.matmul(
                        out=psimT[0:w, i : i + 1],
                        lhsT=dtile[:, o + dc * w : o + (dc + 1) * w],
                        rhs=qt_sb[:, dc * BL + i : dc * BL + i + 1],
                        start=(dc == 0),
                        stop=(dc == NCH - 1),
                    )

        # ---- epilogue on [b=128, m=128] fp32 tiles ----
        # full transpose psimT -> sim_raw via 16 DVE 32x32 block transposes
        SQ = 32
        sim_raw = small.tile([BL, M], f32)
        for bi in range(M // SQ):
            for bj in range(BL // SQ):
                nc.vector.transpose(
                    out=sim_raw[bj * SQ : (bj + 1) * SQ, bi * SQ : (bi + 1) * SQ],
                    in_=psimT[bi * SQ : (bi + 1) * SQ, bj * SQ : (bj + 1) * SQ],
                )
        sim_bm = small.tile([BL, M], f32)
        nc.vector.tensor_scalar(sim_bm, sim_raw, 1.0 / TEMP, None, op0=ALU.mult)
        simm = small.tile([BL, M], f32)
        nc.vector.select(simm, mask_sb, sim_bm, negt)

        nmx = small.tile([BL, 1], f32)
        nc.vector.tensor_reduce(nmx, simm, axis=AX.X, op=ALU.max, negate=True)
        shifted = small.tile([BL, M], f32)
        nc.vector.tensor_scalar_add(shifted, simm, nmx[:, 0:1])

        e_sb = small.tile([BL, M], f32)
        s_sb = small.tile([BL, 1], f32)
        nc.scalar.activation(e_sb, shifted, AF.Exp, accum_out=s_sb)
        logs = small.tile([BL, 1], f32)
        nc.scalar.activation(logs, s_sb, AF.Ln)

        tsum = small.tile([BL, 1], f32)
        nc.vector.tensor_reduce(tsum, traw_sb, axis=AX.X, op=ALU.add)
        denom = small.tile([BL, 1], f32)
        nc.vector.tensor_scalar_add(denom, tsum, 1e-9)
        rec = small.tile([BL, 1], f32)
        nc.vector.reciprocal(rec, denom)
        tn = small.tile([BL, M], f32)
        nc.vector.tensor_scalar_mul(tn, traw_sb, rec[:, 0:1])
        sumtn = small.tile([BL, 1], f32)
        nc.vector.tensor_mul(sumtn, tsum, rec)

        iszero = small.tile([BL, M], f32)
        nc.vector.tensor_scalar(iszero, tn, 0.0, None, op0=ALU.is_le)
        tsafe = small.tile([BL, M], f32)
        nc.vector.tensor_add(tsafe, tn, iszero)
        logt = small.tile([BL, M], f32)
        nc.scalar.activation(logt, tsafe, AF.Ln)

        sc2 = small.tile([BL, M], f32)
        term1 = small.tile([BL, 1], f32)
        nc.vector.scalar_tensor_tensor(
            out=sc2, in0=tn, scalar=1.0, in1=logt,
            op0=ALU.mult, op1=ALU.mult, accum_out=term1,
        )
        sc3 = small.tile([BL, M], f32)
        term2 = small.tile([BL, 1], f32)
        nc.vector.scalar_tensor_tensor(
            out=sc3, in0=tn, scalar=1.0, in1=shifted,
            op0=ALU.mult, op1=ALU.mult, accum_out=term2,
        )

        lgs = small.tile([BL, 1], f32)
        nc.vector.tensor_mul(lgs, logs, sumtn)
        kc = small.tile([BL, 1], f32)
        nc.vector.tensor_sub(kc, term1, term2)
        nc.vector.tensor_add(kc, kc, lgs)

        tot = small.tile([128, 1], f32)
        nc.gpsimd.partition_all_reduce(
            tot, kc, channels=128, reduce_op=bass_isa.ReduceOp.add
        )
        nc.sync.dma_start(out=out, in_=tot[0:1, 0:1])

    nc.compile()
    return nc


def _get_nc(**cfg):
    key = ("nc",) + tuple(sorted(cfg.items()))
    if key not in _CACHE:
        c = dict(cfg)
        if "widths" in c:
            _CACHE[key] = _build_nc_pe(**c)
        else:
            _CACHE[key] = _build_nc(**c)
    return _CACHE[key]


def _make_in_maps_pe(query_embeds, doc_embeds, soft_labels, num_docs_per_sample):
    """Sort samples by doc count, deal round-robin to cores so slot i holds
    near-equal nd everywhere; slot width = max nd over cores at that slot.
    Docs packed per core as a [128 dpos, TOT] bf16 slab: per slot, 8
    transposed d-chunk tiles [128, w] back to back."""
    import ml_dtypes

    bf16 = ml_dtypes.bfloat16
    NCH = D // 128
    qf = np.asarray(query_embeds, dtype=np.float32)
    de = np.asarray(doc_embeds, dtype=np.float32)
    sl = np.ascontiguousarray(np.asarray(soft_labels, dtype=np.float32))
    nd = np.asarray(num_docs_per_sample).astype(np.int64)
    total = de.shape[0]

    offs = np.zeros(B, np.int64)
    offs[1:] = np.cumsum(nd)[:-1]
    nde = np.minimum(np.minimum(nd, M), np.maximum(total - offs, 0))
    maskf = (np.arange(M)[None, :] < nde[:, None]).astype(np.float32)
    traw = sl * maskf

    order = np.argsort(nde, kind="stable")  # ascending nd
    # slot i on core c processes sample order[i*NCORES + c]
    sl_mat = order.reshape(BL, NCORES)
    widths = nde[sl_mat].max(axis=1).astype(np.int64)  # [BL], static per slot
    woffs = np.zeros(BL + 1, np.int64)
    woffs[1:] = np.cumsum(NCH * widths)
    TOT = int(woffs[-1])

    de16 = de.astype(bf16)
    q16 = qf.astype(bf16)

    in_maps = []
    for c in range(NCORES):
        samp = sl_mat[:, c]  # [BL] sample ids for this core, slot order
        slab = np.zeros((128, TOT), bf16)
        qt = np.zeros((128, NCH * BL), bf16)
        for i in range(BL):
            s = int(samp[i])
            n = int(nde[s])
            w = int(widths[i])
            o = int(offs[s])
            # [n, D] -> [D, n] -> [NCH, 128, n]
            t3 = np.ascontiguousarray(de16[o : o + n].T).reshape(NCH, 128, n)
            base = int(woffs[i])
            for dc in range(NCH):
                slab[:, base + dc * w : base + dc * w + n] = t3[dc]
            qt[:, i::BL] = q16[s].reshape(NCH, 128).T  # qt[dpos, dc*BL+i]
        in_maps.append(
            {
                "docs": slab,
                "qt": qt,
                "traw": np.ascontiguousarray(traw[samp]),
                "mask": np.ascontiguousarray(maskf[samp].astype(np.uint8)),
            }
        )
    return in_maps, {"widths": tuple(int(w) for w in widths)}


USE_PE = True


def _make_in_maps(query_embeds, doc_embeds, soft_labels, num_docs_per_sample):
    if USE_PE:
        return _make_in_maps_pe(
            query_embeds, doc_embeds, soft_labels, num_docs_per_sample
        )
    return _make_in_maps_dense(
        query_embeds, doc_embeds, soft_labels, num_docs_per_sample
    )


def _make_in_maps_dense(query_embeds, doc_embeds, soft_labels, num_docs_per_sample):
    import ml_dtypes

    bf16 = ml_dtypes.bfloat16
    qf = np.asarray(query_embeds, dtype=np.float32)
    de = np.asarray(doc_embeds, dtype=np.float32)
    sl = np.ascontiguousarray(np.asarray(soft_labels, dtype=np.float32))
    nd = np.asarray(num_docs_per_sample).astype(np.int64)
    total = de.shape[0]

    offs = np.zeros(B, np.int64)
    offs[1:] = np.cumsum(nd)[:-1]
    # effective (clipped) doc counts, mirroring the reference's clip behaviour
    nde = np.minimum(np.minimum(nd, M), np.maximum(total - offs, 0))
    mask = (np.arange(M)[None, :] < nde[:, None]).astype(np.float32)
    traw = sl * mask

    docs_pad = np.zeros((B, M, D), bf16)
    de16 = de.astype(bf16)
    for b in range(B):
        n = int(nde[b])
        o = int(offs[b])
        if n > 0:
            docs_pad[b, :n] = de16[o : o + n]
    q16 = qf.astype(bf16)

    in_maps = []
    for c in range(NCORES):
        s = slice(c * BL, (c + 1) * BL)
        in_maps.append(
            {
                "docs": np.ascontiguousarray(docs_pad[s].reshape(BL * M, D)),
                "q": np.ascontiguousarray(q16[s]),
                "traw": np.ascontiguousarray(traw[s]),
                "mask": np.ascontiguousarray(mask[s].astype(np.uint8)),
            }
        )
    return in_maps, {}


def run(in_maps, cfg=None, trace=False):
    from concourse import bass_utils

    nc = _get_nc(**(cfg or {}))
    return bass_utils.run_bass_kernel_spmd(
        nc, in_maps, list(range(NCORES)), trace=trace
    )


def kernel(query_embeds, doc_embeds, soft_labels, num_docs_per_sample):
    in_maps, cfg = _make_in_maps(
        query_embeds, doc_embeds, soft_labels, num_docs_per_sample
    )
    res = run(in_maps, cfg=cfg)
    tot = sum(float(r["out"][0, 0]) for r in res.results)
    return np.asarray(tot / B, dtype=np.float32)
